# revision 10
# baseline (speedup 1.0000x reference)
"""Distributed Trainium2 kernel for a single causal attention head with RoPE.

Problem: x [2, 4096, 4096] f32, Wq/Wk/Wv [128, 4096] f32 ->
         out [2, 4096, 128] f32   (causal softmax(q k^T / sqrt(128)) v)

Strategy (8 NeuronCores, 2 groups of 4):
  - core i: batch b = i//4, group rank j = i%4
  - core owns rows {j + 4*m : m in [0, 1024)} of its batch (stride-4
    interleave) -> every core has an IDENTICAL causal work shape (the
    per-core differences live in the data: x rows, RoPE tables, masks)
  - computes q,k,v for its rows (bf16 matmuls, f32 PSUM), applies RoPE
    to q,k, AllGathers k^T then v (+ones column) within its group, then
    computes attention for its 1024 queries over the causal prefix.
  - softmax without max-subtraction (scores ~ N(0,1), exp is safe in
    f32) -> per-rank partial sums are plain sums; denominator comes for
    free from an appended ones-column in v.
  - all inputs are host-pre-tiled so every DMA moves >=2KB contiguous
    runs in few instructions (HWDGE descriptor-gen is ~0.6us/DMA).
"""

import math

import numpy as np
import ml_dtypes

import concourse.bass as bass
import concourse.mybir as mybir
import concourse.tile as tile
from concourse import bacc

BF16 = mybir.dt.bfloat16
F32 = mybir.dt.float32
H = 128            # head dim
GS = 4             # cores per replica group (one batch per group)
N_CORES = 8
GROUPS = [[0, 1, 2, 3], [4, 5, 6, 7]]


def _split_groups(nk, gk):
    """Split the k-chunk loop [0, nk) into exp batches of <= gk chunks,
    keeping the final batch >= 4 so the 4 diagonal chunks stay together."""
    sizes = []
    rem = nk
    while rem > 0:
        g = min(gk, rem)
        if 0 < rem - g < 4:
            g = rem - 4
        sizes.append(g)
        rem -= g
    assert sum(sizes) == nk and sizes[-1] >= 4, (nk, gk, sizes)
    return sizes


def build_nc(S=4096, D=4096, gk=12):
    R = S // GS                 # rows per core
    ND = D // 128               # contraction chunks
    CD = min(4, ND)             # d-chunks per xt DMA chunk
    NXC = ND // CD              # xt DMA chunks
    NT = R // 128               # query blocks per core
    PW = min(512, R)            # psum tile width for projections
    NH = R // PW                # projection psum tiles per tensor
    NSUB = R // 128             # 128-row v sub-tiles
    KT_ELE = H * R              # k^T elements per rank
    V_ELE = R * (H + 1)         # v (+ones) elements per rank
    SCALE = 1.0 / math.sqrt(H)

    nc = bacc.Bacc(None, target_bir_lowering=False)

    # host-pre-tiled inputs (see make_core_inputs)
    xt = nc.dram_tensor("xt", [128, ND * R], BF16, kind="ExternalInput")
    wq = nc.dram_tensor("wq", [128, ND * H], BF16, kind="ExternalInput")
    wk = nc.dram_tensor("wk", [128, ND * H], BF16, kind="ExternalInput")
    wv = nc.dram_tensor("wv", [128, ND * H], BF16, kind="ExternalInput")
    cos_in = nc.dram_tensor("cos", [H, R], F32, kind="ExternalInput")
    sin_in = nc.dram_tensor("sin", [H, R], F32, kind="ExternalInput")
    smat_in = nc.dram_tensor("smat", [H, H], BF16, kind="ExternalInput")
    masks_in = nc.dram_tensor("masks", [128, 4 * 128], BF16, kind="ExternalInput")
    out = nc.dram_tensor("out", [R, H], F32, kind="ExternalOutput")

    with tile.TileContext(nc) as tc:
        with tc.tile_pool(name="dram", bufs=1, space="DRAM") as dram, \
             tc.tile_pool(name="const", bufs=1) as constp, \
             tc.tile_pool(name="xpool", bufs=1) as xpool, \
             tc.tile_pool(name="work", bufs=1) as work:

            k_loc = dram.tile([KT_ELE], BF16)
            k_g = dram.tile([GS, KT_ELE], BF16)
            v_loc = dram.tile([V_ELE], BF16)
            v_g = dram.tile([GS, V_ELE], BF16)

            # ---- input DMAs (order matters: wk/wq + first xt chunks gate PE) ----
            wk_t = constp.tile([128, ND * H], BF16, name="wk_t")
            nc.sync.dma_start(out=wk_t[:], in_=wk[:])
            wq_t = constp.tile([128, ND * H], BF16, name="wq_t")
            nc.sync.dma_start(out=wq_t[:], in_=wq[:])
            xts = []
            for g in range(NXC):
                xtile = xpool.tile([128, CD * R], BF16, name=f"xt_{g}")
                nc.sync.dma_start(out=xtile[:],
                                  in_=xt[:, CD * R * g:CD * R * (g + 1)])
                xts.append(xtile)
            wv_t = constp.tile([128, ND * H], BF16, name="wv_t")
            nc.sync.dma_start(out=wv_t[:], in_=wv[:])
            cos_t = constp.tile([H, R], F32, name="cos_t")
            nc.sync.dma_start(out=cos_t[:], in_=cos_in[:])
            sin_t = constp.tile([H, R], F32, name="sin_t")
            nc.sync.dma_start(out=sin_t[:], in_=sin_in[:])
            smat_t = constp.tile([H, H], BF16, name="smat_t")
            nc.sync.dma_start(out=smat_t[:], in_=smat_in[:])
            mask_t = constp.tile([128, 4 * 128], BF16, name="mask_t")
            nc.sync.dma_start(out=mask_t[:], in_=masks_in[:])

            def xs_d(d):
                """SBUF slice view of x^T d-chunk d: [128, R]."""
                g, o = divmod(d, CD)
                return xts[g][:, R * o:R * (o + 1)]

            def rope(psums, swp_pool, dst_tile, tagp):
                """RoPE a projected tensor ([h, s] layout, NH psum tiles of
                [128, PW] f32) into bf16 dst_tile [128, R]."""
                for hh in range(NH):
                    sl = slice(PW * hh, PW * (hh + 1))
                    raw = work.tile([128, PW], BF16, name=f"rope_raw_{tagp}_{hh}",
                                    tag="rope_raw", bufs=2)
                    nc.vector.tensor_copy(raw[:], psums[hh][:])
                    swp = swp_pool.tile([128, PW], F32, name=f"rope_swp_{tagp}_{hh}",
                                        tag="rope_swp", bufs=2)
                    nc.tensor.matmul(swp[:], lhsT=smat_t[:], rhs=raw[:],
                                     start=True, stop=True)
                    t1 = work.tile([128, PW], F32, name=f"rope_t1_{tagp}_{hh}",
                                   tag="rope_t1", bufs=2)
                    nc.vector.tensor_tensor(t1[:], psums[hh][:], cos_t[:, sl],
                                            mybir.AluOpType.mult)
                    t2 = work.tile([128, PW], F32, name=f"rope_t2_{tagp}_{hh}",
                                   tag="rope_t2", bufs=2)
                    nc.vector.tensor_tensor(t2[:], swp[:], sin_t[:, sl],
                                            mybir.AluOpType.mult)
                    nc.vector.tensor_tensor(dst_tile[:, sl], t1[:], t2[:],
                                            mybir.AluOpType.add)

            # ---- K + Q projections interleaved (fills the xt DMA window),
            #      then RoPE K -> AllGather K, V -> AllGather V, RoPE Q ----
            kT_sb = work.tile([128, R], BF16, name="kT_sb")
            qT_sb = work.tile([128, R], BF16, name="qT_sb")
            v_all_loc = work.tile([128, NSUB * (H + 1)], BF16, name="v_all_loc")
            with tc.tile_pool(name="psum_proj", bufs=1, space="PSUM") as pp:
                k_ps = [pp.tile([128, PW], F32, name=f"k_ps{h}") for h in range(NH)]
                q_ps = [pp.tile([128, PW], F32, name=f"q_ps{h}") for h in range(NH)]
                for d in range(ND):
                    for hh in range(NH):
                        nc.tensor.matmul(k_ps[hh][:],
                                         lhsT=wk_t[:, H * d:H * (d + 1)],
                                         rhs=xs_d(d)[:, PW * hh:PW * (hh + 1)],
                                         start=(d == 0), stop=(d == ND - 1))
                    for hh in range(NH):
                        nc.tensor.matmul(q_ps[hh][:],
                                         lhsT=wq_t[:, H * d:H * (d + 1)],
                                         rhs=xs_d(d)[:, PW * hh:PW * (hh + 1)],
                                         start=(d == 0), stop=(d == ND - 1))
                rope(k_ps, pp, kT_sb, "k")
                nc.sync.dma_start(out=k_loc[:].rearrange("(p f) -> p f", p=128),
                                  in_=kT_sb[:])
                nc.gpsimd.collective_compute(
                    "AllGather", mybir.AluOpType.bypass, replica_groups=GROUPS,
                    ins=[k_loc[:].opt()], outs=[k_g[:].opt()])

                # V projection (sub-major; each accumulation owns its bank)
                for sub in range(NSUB):
                    v_ps = pp.tile([128, 128], F32, name=f"v_ps_{sub}",
                                   tag="v_ps", bufs=2)
                    for d in range(ND):
                        nc.tensor.matmul(v_ps[:],
                                         lhsT=xs_d(d)[:, 128 * sub:128 * (sub + 1)],
                                         rhs=wv_t[:, H * d:H * (d + 1)],
                                         start=(d == 0), stop=(d == ND - 1))
                    co = (H + 1) * sub
                    nc.vector.tensor_copy(v_all_loc[:, co:co + H], v_ps[:])
                ones_view = v_all_loc[:].rearrange("p (s h) -> p s h", h=H + 1)[:, :, H]
                nc.vector.memset(ones_view, 1.0)
                nc.sync.dma_start(
                    out=v_loc[:].rearrange("(s p h) -> p s h", p=128, h=H + 1),
                    in_=v_all_loc[:].rearrange("p (s h) -> p s h", h=H + 1))
                nc.gpsimd.collective_compute(
                    "AllGather", mybir.AluOpType.bypass, replica_groups=GROUPS,
                    ins=[v_loc[:].opt()], outs=[v_g[:].opt()])

                rope(q_ps, pp, qT_sb, "q")

            # ---- load gathered K^T and V into SBUF ----
            # key chunk c = 4*lc + r: rank r's local rows [128*lc, 128*(lc+1))
            ktf = work.tile([128, GS * R], BF16, name="ktf")
            for r in range(GS):
                nc.sync.dma_start(
                    out=ktf[:, R * r:R * (r + 1)],
                    in_=k_g[r, :].rearrange("(p f) -> p f", p=128))
            v_ts = []
            for r in range(GS):
                v_t = work.tile([128, (R // 128) * (H + 1)], BF16, name=f"v_t{r}")
                nc.sync.dma_start(
                    out=v_t[:],
                    in_=v_g[r, :].rearrange("(lc p h) -> p lc h", p=128, h=H + 1))
                v_ts.append(v_t)

            def k_chunk(c):
                lc, r = divmod(c, GS)
                return ktf[:, R * r + 128 * lc:R * r + 128 * (lc + 1)]

            def v_chunk(c):
                lc, r = divmod(c, GS)
                return v_ts[r][:, (H + 1) * lc:(H + 1) * (lc + 1)]

            # ---- attention ----
            o_all = work.tile([128, NT * H], F32, name="o_all")
            with tc.tile_pool(name="psum_sc", bufs=2, space="PSUM") as psc, \
                 tc.tile_pool(name="psum_o", bufs=2, space="PSUM") as pso:
                for t in range(NT):
                    nk = 4 * t + 4
                    out_aug = pso.tile([128, H + 1], F32, name=f"oaug_{t}",
                                       tag="oaug")
                    cstart = 0
                    for gsize in _split_groups(nk, gk):
                        sc = psc.tile([128, 128 * gk], F32, name=f"sc_{t}_{cstart}",
                                      tag="sc")
                        for ci in range(gsize):
                            c = cstart + ci
                            nc.tensor.matmul(
                                sc[:, 128 * ci:128 * (ci + 1)],
                                lhsT=k_chunk(c),
                                rhs=qT_sb[:, 128 * t:128 * (t + 1)],
                                start=True, stop=True)
                        pb = work.tile([128, 128 * gk], BF16,
                                       name=f"pb_{t}_{cstart}", tag="pb", bufs=2)
                        nc.scalar.activation(pb[:, :128 * gsize],
                                             sc[:, :128 * gsize],
                                             mybir.ActivationFunctionType.Exp,
                                             scale=SCALE)
                        if cstart + gsize == nk:  # group with the diagonal chunks
                            dsl = slice(128 * (gsize - 4), 128 * gsize)
                            nc.vector.tensor_tensor(pb[:, dsl], pb[:, dsl],
                                                    mask_t[:],
                                                    mybir.AluOpType.mult)
                        for ci in range(gsize):
                            c = cstart + ci
                            nc.tensor.matmul(
                                out_aug[:],
                                lhsT=pb[:, 128 * ci:128 * (ci + 1)],
                                rhs=v_chunk(c),
                                start=(c == 0), stop=(c == nk - 1))
                        cstart += gsize
                    recip = work.tile([128, 1], F32, name=f"recip_{t}",
                                      tag="recip", bufs=2)
                    nc.vector.reciprocal(recip[:], out_aug[:, H:H + 1])
                    nc.vector.tensor_scalar(o_all[:, H * t:H * (t + 1)],
                                            out_aug[:, :H], recip[:],
                                            None, mybir.AluOpType.mult)
            nc.sync.dma_start(out=out[:].rearrange("(t p) h -> p t h", p=128),
                              in_=o_all[:].rearrange("p (t h) -> p t h", h=H))

    nc.finalize()
    return nc


# ---------------------------------------------------------------------------
# host side
# ---------------------------------------------------------------------------

def make_core_inputs(x, Wq, Wk, Wv, core_id, S, D):
    """Build the per-core input dict (numpy) for core `core_id`."""
    R = S // GS
    ND = D // 128
    b, j = divmod(core_id, GS)
    bf16 = ml_dtypes.bfloat16
    xs = x[b, j::GS, :]                      # [R, D]
    # pre-tiled x^T: xt[p, n*R + m] = xs[m, 128n + p]
    xt = np.ascontiguousarray(
        xs.reshape(R, ND, 128).transpose(2, 1, 0)).reshape(128, ND * R)

    def tile_w(W):
        # w[p, n*H + h] = W[h, 128n + p]
        return np.ascontiguousarray(
            W.T.reshape(ND, 128, H).transpose(1, 0, 2)).reshape(128, ND * H) \
            .astype(bf16)

    # RoPE tables for this core's positions (pairs duplicated, sign folded)
    pos = (j + GS * np.arange(R)).astype(np.float64)
    idx = np.arange(0, H, 2, dtype=np.float64)
    theta = np.power(10000.0, -2.0 * idx / H)         # [64]
    ang = pos[None, :] * theta[:, None]               # [64, R]
    cos = np.empty((H, R), np.float32)
    sin = np.empty((H, R), np.float32)
    cos[0::2, :] = np.cos(ang)
    cos[1::2, :] = np.cos(ang)
    sin[0::2, :] = -np.sin(ang)
    sin[1::2, :] = np.sin(ang)
    # pair-swap matrix (signs are folded into `sin` above)
    smat = np.zeros((H, H), np.float32)
    smat[np.arange(0, H, 2), np.arange(1, H, 2)] = 1.0
    smat[np.arange(1, H, 2), np.arange(0, H, 2)] = 1.0
    # diagonal masks: chunk (lc=t, r): sk row kk holds global key
    # 512t + r + 4*kk vs query 512t + j + 4*qq -> allow r + 4*kk <= j + 4*qq
    kk = np.arange(128)
    qq = np.arange(128)
    masks = np.empty((128, 4 * 128), np.float32)
    for r in range(4):
        masks[:, 128 * r:128 * (r + 1)] = \
            ((r + 4 * kk)[:, None] <= (j + 4 * qq)[None, :]).astype(np.float32)
    return {
        "xt": xt.astype(bf16),
        "wq": tile_w(Wq),
        "wk": tile_w(Wk),
        "wv": tile_w(Wv),
        "cos": cos,
        "sin": sin,
        "smat": smat.astype(bf16),
        "masks": masks.astype(bf16),
    }


def assemble_output(results, B, S):
    R = S // GS
    out = np.empty((B, S, H), np.float32)
    for i in range(B * GS):
        b, j = divmod(i, GS)
        out[b, j::GS, :] = results[i]["out"]
    return out


class _SpmdRunner:
    """Minimal SPMD runner mirroring bass2jax.run_bass_via_pjrt but keeping
    the jitted executable for reuse."""

    def __init__(self, nc, n_cores=N_CORES):
        import jax
        from jax.sharding import Mesh, PartitionSpec
        from jax.experimental.shard_map import shard_map
        from concourse.bass2jax import (_bass_exec_p, install_neuronx_cc_hook,
                                        partition_id_tensor)
        install_neuronx_cc_hook()
        self.jax = jax
        self.n_cores = n_cores
        pname = nc.partition_id_tensor.name if nc.partition_id_tensor else None
        in_names, out_names, out_avals, zero_outs = [], [], [], []
        for alloc in nc.m.functions[0].allocations:
            if not isinstance(alloc, mybir.MemoryLocationSet):
                continue
            name = alloc.memorylocations[0].name
            if alloc.kind == "ExternalInput":
                if name != pname:
                    in_names.append(name)
            elif alloc.kind == "ExternalOutput":
                shape = tuple(alloc.tensor_shape)
                dtype = mybir.dt.np(alloc.dtype)
                out_names.append(name)
                out_avals.append(jax.core.ShapedArray(shape, dtype))
                zero_outs.append(np.zeros(shape, dtype))
        self.n_params = len(in_names)
        all_in = list(in_names) + list(out_names)
        if pname is not None:
            all_in.append(pname)

        def _body(*args):
            operands = list(args)
            if pname is not None:
                operands.append(partition_id_tensor())
            outs = _bass_exec_p.bind(
                *operands, out_avals=tuple(out_avals), in_names=tuple(all_in),
                out_names=tuple(out_names), lowering_input_output_aliases=(),
                sim_require_finite=True, sim_require_nnan=True, nc=nc)
            return tuple(outs)

        devices = jax.devices()[:n_cores]
        mesh = Mesh(np.asarray(devices), ("core",))
        nio = self.n_params + len(out_names)
        self.sharded = jax.jit(
            shard_map(_body, mesh=mesh,
                      in_specs=(PartitionSpec("core"),) * nio,
                      out_specs=(PartitionSpec("core"),) * len(out_names),
                      check_rep=False),
            keep_unused=True)
        self.in_names, self.out_names, self.out_avals = in_names, out_names, out_avals
        self.zero_outs = zero_outs

    def prep(self, in_maps):
        per_core = [[np.asarray(m[n]) for n in self.in_names] for m in in_maps]
        concat_in = [np.concatenate([per_core[c][i] for c in range(self.n_cores)],
                                    axis=0) for i in range(self.n_params)]
        concat_zero = [np.zeros((self.n_cores * z.shape[0], *z.shape[1:]), z.dtype)
                       for z in self.zero_outs]
        return concat_in + concat_zero

    def run(self, args):
        out_arrs = self.sharded(*args)
        self.jax.block_until_ready(out_arrs)
        return [
            {n: np.asarray(out_arrs[i]).reshape(self.n_cores,
                                                *self.out_avals[i].shape)[c]
             for i, n in enumerate(self.out_names)}
            for c in range(self.n_cores)
        ]


_CACHE = {}


def kernel(x, Wq, Wk, Wv):
    x = np.asarray(x)
    Wq, Wk, Wv = np.asarray(Wq), np.asarray(Wk), np.asarray(Wv)
    B, S, D = x.shape
    assert (B, S, D) == (2, 4096, 4096) and Wq.shape == (H, D)
    key = (S, D)
    if key not in _CACHE:
        nc = build_nc(S=S, D=D)
        _CACHE[key] = _SpmdRunner(nc)
    runner = _CACHE[key]
    in_maps = [make_core_inputs(x, Wq, Wk, Wv, i, S, D) for i in range(N_CORES)]
    results = runner.run(runner.prep(in_maps))
    return assemble_output(results, B, S)
